# revision 11
# baseline (speedup 1.0000x reference)
"""VQ codebook lookup (DiscreteDecisionEngine) on 8 TRN2 NeuronCores.

Math: for each token x_t, find argmin_k ||x_t - c_k||^2, emit codebook[k].
argmin_k ||x-c||^2 == argmax_k (2*x.c_k - ||c_k||^2)  (||x||^2 constant per token).

Device strategy (data-parallel over tokens, codebook replicated per core):
  - Token tile = 128 tokens. Scores for 8192 codes per tile computed as 4
    PSUM "quarters" of 2048 codes.
  - PE float32r (TF32-like, RNE to 11 mantissa bits, 1 cycle/row) matmuls:
    score = x @ (2C)^T - ||c||^2, with the ||c||^2 term folded in as a 5th
    K=2 contraction step (ones x [-csq_hi; -csq_lo] split keeps csq exact to
    ~3e-5 despite f32r rounding).
  - DVE reduce_max per quarter directly on PSUM -> qmax.
  - ACT drains PSUM -> SBUF score tiles (idle engine otherwise).
  - tau = global max - DELTA band. Pass 2 (scalar_tensor_tensor, DVE 2x_2p):
    S_q = sum_k (score >= tau) * (BPACK + k_local).
  - decode: count_q = S_q div BPACK; exactly one in-band code => exact index;
    total count emitted as a per-token flag.
  - GPSIMD indirect DMA gathers codebook rows, HWDGE stores output.

Host: tokens whose flag != 1 (a second code within DELTA of the max -- f32r
rounding could misrank those) are recomputed exactly in float64. Device score
error vs exact fp32 is bounded by ~0.07 (11-bit input rounding over D=512),
so DELTA=0.2 is sound with ~3x margin; ~1-2% of tokens get flagged.
"""

import numpy as np

import concourse.bacc as bacc
import concourse.bass as bass
import concourse.mybir as mybir
from concourse.tile import TileContext

P = 128          # partitions / token tile
D = 512          # latent dim
K = 8192         # codebook size
N_TOKENS = 32768
N_CORES = 8
T_PER_CORE = N_TOKENS // N_CORES   # 4096
N_TILES_FULL = T_PER_CORE // P     # 32
QUARTER_FULL = 2048                # codes per PSUM quarter (4 banks)
N_CHUNK = D // P                   # 4 contraction chunks
MM_N = 512                         # matmul free-dim block (1 PSUM bank, fp32)

F32 = mybir.dt.float32
F32R = mybir.dt.float32r
BPACK = float(1 << 17)             # count-packing base in pass 2
DELTA = 0.2                        # at-risk band below the device max
MANT = 11                          # f32r = RNE to 11 explicit mantissa bits


def build_bass(n_tiles=N_TILES_FULL, k=K, quarter=QUARTER_FULL):
    """Build the single-core Bass program (SPMD across cores)."""
    n_q = k // quarter
    n_sb = max(1, quarter // MM_N)
    sb = min(MM_N, quarter)

    nc = bacc.Bacc()
    x_tiles = nc.declare_dram_parameter(
        "x_tiles", [n_tiles, P, N_CHUNK, P], F32R, isOutput=False)
    cb_tiles = nc.declare_dram_parameter(
        "cb_tiles", [N_CHUNK, n_q, P, quarter], F32R, isOutput=False)
    # rows 2q / 2q+1 hold -csq_hi / -csq_lo for quarter q; the per-quarter
    # K=8 selector weight (0/1 rows) picks the right pair so every matmul
    # anchors at base partition 0.
    csqpack = nc.declare_dram_parameter("csqpack", [2 * n_q, quarter], F32R,
                                        isOutput=False)
    selrows = nc.declare_dram_parameter("selrows", [n_q, 2 * n_q, P], F32R,
                                        isOutput=False)
    iota_b = nc.declare_dram_parameter("iota_b", [P, quarter], F32,
                                       isOutput=False)
    iota_nq = nc.declare_dram_parameter("iota_nq", [P, n_q], F32,
                                        isOutput=False)
    codebook = nc.declare_dram_parameter("codebook", [k, D], F32,
                                         isOutput=False)
    out = nc.declare_dram_parameter("out", [n_tiles * P, D], F32,
                                    isOutput=True)
    out_flags = nc.declare_dram_parameter(
        "out_flags", [P, n_tiles], F32, isOutput=True)

    with TileContext(nc) as tc:
        with (
            tc.tile_pool(name="const", bufs=1) as cpool,
            tc.tile_pool(name="xp", bufs=3) as xpool,
            tc.tile_pool(name="sp", bufs=5) as spool,
            tc.tile_pool(name="small", bufs=2) as smpool,
            tc.tile_pool(name="sm1", bufs=1) as sm1pool,
            tc.tile_pool(name="op", bufs=2) as opool,
            tc.tile_pool(name="ps", bufs=2, space="PSUM") as pspool,
        ):
            # --- resident constants ------------------------------------------
            cb_sb = {}
            for c in range(N_CHUNK):
                for q in range(n_q):
                    t = cpool.tile([P, quarter], F32R, tag=f"cb_{c}_{q}")
                    nc.sync.dma_start(out=t, in_=cb_tiles[c, q])
                    cb_sb[c, q] = t
            csq_sb = cpool.tile([2 * n_q, quarter], F32R, tag="csqpack")
            nc.sync.dma_start(out=csq_sb, in_=csqpack[:, :])
            sel_sb = {}
            for q in range(n_q):
                st = cpool.tile([2 * n_q, P], F32R, tag=f"sel_{q}")
                nc.sync.dma_start(out=st, in_=selrows[q])
                sel_sb[q] = st
            iota_sb = cpool.tile([P, quarter], F32, tag="iota")
            nc.sync.dma_start(out=iota_sb, in_=iota_b[:, :])
            iota_nq_sb = cpool.tile([P, n_q], F32, tag="iota_nq")
            nc.sync.dma_start(out=iota_nq_sb, in_=iota_nq[:, :])
            flags_sb = cpool.tile([P, n_tiles], F32, tag="flags")

            # --- main loop over token tiles ----------------------------------
            for tt in range(n_tiles):
                xt = xpool.tile([P, N_CHUNK, P], F32R, tag="xt")
                nc.sync.dma_start(out=xt, in_=x_tiles[tt])

                qmax = smpool.tile([P, n_q], F32, tag="qmax")
                sq4 = smpool.tile([P, n_q], F32, tag="sq4")
                scores = []

                for q in range(n_q):
                    ps = pspool.tile([P, quarter], F32, tag="ps")
                    for c in range(N_CHUNK):
                        for s in range(n_sb):
                            nc.tensor.matmul(
                                out=ps[:, s * sb:(s + 1) * sb],
                                lhsT=xt[:, c, :],
                                rhs=cb_sb[c, q][:, s * sb:(s + 1) * sb],
                                start=(c == 0),
                                stop=False,
                            )
                    for s in range(n_sb):
                        nc.tensor.matmul(
                            out=ps[:, s * sb:(s + 1) * sb],
                            lhsT=sel_sb[q][:, :],
                            rhs=csq_sb[:, s * sb:(s + 1) * sb],
                            start=False,
                            stop=True,
                        )
                    # per-quarter max straight off PSUM (DVE)
                    nc.vector.reduce_max(
                        out=qmax[:, q:q + 1], in_=ps,
                        axis=mybir.AxisListType.X)
                    # drain scores PSUM -> SBUF on the idle ACT engine
                    score = spool.tile([P, quarter], F32, tag="score")
                    nc.scalar.copy(score, ps)
                    scores.append(score)

                # tau = gmax - DELTA
                gmax = sm1pool.tile([P, 1], F32, tag="gmax")
                nc.vector.reduce_max(
                    out=gmax, in_=qmax, axis=mybir.AxisListType.X)
                tau = sm1pool.tile([P, 1], F32, tag="tau")
                nc.vector.tensor_scalar_add(tau, gmax, -DELTA)

                # pass 2: S_q = sum_k (score_k >= tau) * (BPACK + k_local)
                # (all-SBUF operands -> DVE 2x_2p mode, 0.5 cyc/el)
                for q in range(n_q):
                    dummy = sm1pool.tile([P, 1], F32, tag=f"dummy{min(q, 1)}")
                    nc.vector.scalar_tensor_tensor(
                        out=dummy.broadcast_to((P, quarter)),
                        in0=scores[q],
                        scalar=tau,
                        in1=iota_sb,
                        op0=mybir.AluOpType.is_ge,
                        op1=mybir.AluOpType.mult,
                        accum_out=sq4[:, q:q + 1],
                    )

                # decode: count = floor(S/B) via f32->u32 convert (frac < 2^-6);
                # idx_local = S - count*B
                t1 = sm1pool.tile([P, n_q], F32, tag="t1")
                nc.vector.tensor_scalar_mul(t1, sq4, 1.0 / BPACK)
                cnt_u = sm1pool.tile([P, n_q], mybir.dt.uint32, tag="cnt_u")
                nc.vector.tensor_copy(cnt_u, t1)
                count4 = sm1pool.tile([P, n_q], F32, tag="count4")
                nc.vector.tensor_copy(count4, cnt_u)
                cntb = sm1pool.tile([P, n_q], F32, tag="cntb")
                nc.vector.tensor_scalar_mul(cntb, count4, BPACK)
                idx_local = sm1pool.tile([P, n_q], F32, tag="idx_local")
                nc.vector.tensor_sub(idx_local, sq4, cntb)
                nc.vector.reduce_sum(
                    out=flags_sb[:, tt:tt + 1], in_=count4,
                    axis=mybir.AxisListType.X)
                # global candidate index per quarter; select in-band quarters
                idxg = sm1pool.tile([P, n_q], F32, tag="idxg")
                nc.vector.tensor_add(idxg, idx_local, iota_nq_sb)
                idxf = sm1pool.tile([P, 1], F32, tag="idxf")
                dsel = sm1pool.tile([P, 1], F32, tag="dsel")
                nc.vector.scalar_tensor_tensor(
                    out=dsel.broadcast_to((P, n_q)),
                    in0=count4,
                    scalar=0.5,
                    in1=idxg,
                    op0=mybir.AluOpType.is_ge,
                    op1=mybir.AluOpType.mult,
                    accum_out=idxf,
                )
                idxc = sm1pool.tile([P, 1], F32, tag="idxc")
                nc.vector.tensor_scalar_min(idxc, idxf, float(k - 1))
                idxu = sm1pool.tile([P, 1], mybir.dt.uint32, tag="idxu")
                nc.vector.tensor_copy(idxu, idxc)

                # gather codebook rows and store
                rows = opool.tile([P, D], F32, tag="rows")
                nc.gpsimd.indirect_dma_start(
                    out=rows,
                    out_offset=None,
                    in_=codebook[:, :],
                    in_offset=bass.IndirectOffsetOnAxis(ap=idxu[:, :1], axis=0),
                )
                nc.sync.dma_start(out=out[tt * P:(tt + 1) * P, :], in_=rows)

            nc.sync.dma_start(out=out_flags[:, :], in_=flags_sb)

    return nc


def tf32_round(a, mant=MANT):
    """Round fp32 to `mant` explicit mantissa bits (round-to-nearest)."""
    ai = a.view(np.int32).astype(np.int64)
    shift = 23 - mant
    bias = 1 << (shift - 1)
    r = ((ai + bias) >> shift) << shift
    return r.astype(np.int32).view(np.float32)


def prep_core_inputs(x_core, shared, n_tiles):
    """Per-core input map. x_core: [n_tiles*P, D]."""
    xt = tf32_round(np.ascontiguousarray(
        x_core.reshape(n_tiles, P, N_CHUNK, P).transpose(0, 3, 2, 1)))
    return {"x_tiles": xt, **shared}


def prep_shared(codebook, k, quarter):
    n_q = k // quarter
    cb = np.ascontiguousarray(np.asarray(codebook, dtype=np.float32))
    cb2 = 2.0 * cb  # exact in fp32
    # cb_tiles[c, q, d, j] = cb2[q*quarter + j, c*128 + d]
    cb2_tiles = tf32_round(np.ascontiguousarray(
        cb2.reshape(n_q, quarter, N_CHUNK, P).transpose(2, 0, 3, 1)))
    csq = (cb * cb).sum(axis=1, dtype=np.float32)
    csqpack = np.zeros((2 * n_q, quarter), dtype=np.float32)
    selrows = np.zeros((n_q, 2 * n_q, P), dtype=np.float32)
    for q in range(n_q):
        seg = csq[q * quarter:(q + 1) * quarter]
        hi = tf32_round(-seg)
        lo = tf32_round(-seg - hi)
        csqpack[2 * q] = hi
        csqpack[2 * q + 1] = lo
        selrows[q, 2 * q, :] = 1.0
        selrows[q, 2 * q + 1, :] = 1.0
    iota_b_np = np.broadcast_to(
        (np.arange(quarter, dtype=np.float32) + BPACK)[None, :],
        (P, quarter)).copy()
    iota_nq_np = np.broadcast_to(
        (np.arange(n_q, dtype=np.float32) * quarter)[None, :], (P, n_q)).copy()
    return {
        "cb_tiles": cb2_tiles,
        "csqpack": csqpack,
        "selrows": selrows,
        "iota_b": iota_b_np,
        "iota_nq": iota_nq_np,
        "codebook": cb,
    }


_NC_CACHE = {}


def _get_nc(key):
    if key not in _NC_CACHE:
        nc = build_bass(*key)
        nc.finalize()
        _NC_CACHE[key] = nc
    return _NC_CACHE[key]


def _host_rescue(out_full, flags_full, x, codebook):
    """Recompute flagged tokens exactly (float64)."""
    bad = np.flatnonzero(flags_full != 1.0)
    if len(bad) == 0:
        return out_full, 0
    xb = x[bad].astype(np.float64)
    cb64 = codebook.astype(np.float64)
    csq = (cb64 * cb64).sum(1)
    sc = 2.0 * (xb @ cb64.T) - csq[None, :]
    idx = sc.argmax(1)
    out_full[bad] = codebook[idx]
    return out_full, len(bad)


def kernel(x, codebook):
    from concourse.bass_utils import run_bass_kernel_spmd

    x = np.ascontiguousarray(np.asarray(x, dtype=np.float32))
    codebook = np.ascontiguousarray(np.asarray(codebook, dtype=np.float32))
    assert x.shape == (N_TOKENS, D) and codebook.shape == (K, D)

    nc = _get_nc((N_TILES_FULL, K, QUARTER_FULL))
    shared = prep_shared(codebook, K, QUARTER_FULL)

    in_maps = []
    for core in range(N_CORES):
        x_core = x[core * T_PER_CORE:(core + 1) * T_PER_CORE]
        in_maps.append(prep_core_inputs(x_core, shared, N_TILES_FULL))

    res = run_bass_kernel_spmd(nc, in_maps, list(range(N_CORES)))
    out_full = np.concatenate(
        [res.results[i]["out"] for i in range(N_CORES)], axis=0)
    # flags: [P, n_tiles] per core; token (core, tt*128+p) at [p, tt]
    flags_full = np.concatenate(
        [np.asarray(res.results[i]["out_flags"]).T.reshape(-1)
         for i in range(N_CORES)])
    out_full, n_rescued = _host_rescue(out_full, flags_full, x, codebook)
    return out_full


# revision 12
# speedup vs baseline: 2.1778x; 2.1778x over previous
"""VQ codebook lookup (DiscreteDecisionEngine) on 8 TRN2 NeuronCores.

Math: for each token x_t, find argmin_k ||x_t - c_k||^2, emit codebook[k].
argmin_k ||x-c||^2 == argmax_k (2*x.c_k - ||c_k||^2)  (||x||^2 constant per token).

Device strategy (data-parallel over tokens, codebook replicated per core):
  - Token tile = 128 tokens. Scores for 8192 codes per tile computed as 4
    PSUM "quarters" of 2048 codes.
  - PE float32r (TF32-like, RNE to 11 mantissa bits, 1 cycle/row) matmuls:
    score = x @ (2C)^T - ||c||^2, with the ||c||^2 term folded in as a 5th
    K=2 contraction step (ones x [-csq_hi; -csq_lo] split keeps csq exact to
    ~3e-5 despite f32r rounding).
  - DVE reduce_max per quarter directly on PSUM -> qmax.
  - ACT drains PSUM -> SBUF score tiles (idle engine otherwise).
  - tau = global max - DELTA band. Pass 2 (scalar_tensor_tensor, DVE 2x_2p):
    S_q = sum_k (score >= tau) * (BPACK + k_local).
  - decode: count_q = S_q div BPACK; exactly one in-band code => exact index;
    total count emitted as a per-token flag.
  - GPSIMD indirect DMA gathers codebook rows, HWDGE stores output.

Host: tokens whose flag != 1 (a second code within DELTA of the max -- f32r
rounding could misrank those) are recomputed exactly in float64. Device score
error vs exact fp32 is bounded by ~0.07 (11-bit input rounding over D=512),
so DELTA=0.2 is sound with ~3x margin; ~1-2% of tokens get flagged.
"""

import numpy as np

import concourse.bacc as bacc
import concourse.bass as bass
import concourse.mybir as mybir
from concourse.tile import TileContext

P = 128          # partitions / token tile
D = 512          # latent dim
K = 8192         # codebook size
N_TOKENS = 32768
N_CORES = 8
T_PER_CORE = N_TOKENS // N_CORES   # 4096
N_TILES_FULL = T_PER_CORE // P     # 32
QUARTER_FULL = 2048                # codes per PSUM quarter (4 banks)
N_CHUNK = D // P                   # 4 contraction chunks
MM_N = 512                         # matmul free-dim block (1 PSUM bank, fp32)

F32 = mybir.dt.float32
F32R = mybir.dt.float32r
BPACK = float(1 << 17)             # count-packing base in pass 2
DELTA = 0.2                        # at-risk band below the device max
MANT = 11                          # f32r = RNE to 11 explicit mantissa bits


def build_bass(n_tiles=N_TILES_FULL, k=K, quarter=QUARTER_FULL, repeat=1):
    """Build the single-core Bass program (SPMD across cores)."""
    n_q = k // quarter
    n_sb = max(1, quarter // MM_N)
    sb = min(MM_N, quarter)

    nc = bacc.Bacc()
    x_tiles = nc.declare_dram_parameter(
        "x_tiles", [n_tiles, P, N_CHUNK, P], F32R, isOutput=False)
    cb_tiles = nc.declare_dram_parameter(
        "cb_tiles", [N_CHUNK, n_q, P, quarter], F32R, isOutput=False)
    # rows 2q / 2q+1 hold -csq_hi / -csq_lo for quarter q; the per-quarter
    # K=8 selector weight (0/1 rows) picks the right pair so every matmul
    # anchors at base partition 0.
    csqpack = nc.declare_dram_parameter("csqpack", [2 * n_q, quarter], F32R,
                                        isOutput=False)
    selrows = nc.declare_dram_parameter("selrows", [n_q, 2 * n_q, P], F32R,
                                        isOutput=False)
    iota_b = nc.declare_dram_parameter("iota_b", [P, quarter], F32,
                                       isOutput=False)
    iota_nq = nc.declare_dram_parameter("iota_nq", [P, n_q], F32,
                                        isOutput=False)
    codebook = nc.declare_dram_parameter("codebook", [k, D], F32,
                                         isOutput=False)
    out = nc.declare_dram_parameter("out", [n_tiles * P, D], F32,
                                    isOutput=True)
    out_flags = nc.declare_dram_parameter(
        "out_flags", [P, n_tiles], F32, isOutput=True)

    with TileContext(nc) as tc:
        with (
            tc.tile_pool(name="const", bufs=1) as cpool,
            tc.tile_pool(name="xp", bufs=3) as xpool,
            tc.tile_pool(name="sp", bufs=5) as spool,
            tc.tile_pool(name="small", bufs=2) as smpool,
            tc.tile_pool(name="sm1", bufs=1) as sm1pool,
            tc.tile_pool(name="op", bufs=2) as opool,
            tc.tile_pool(name="ps", bufs=2, space="PSUM") as pspool,
        ):
            # --- resident constants ------------------------------------------
            cb_sb = {}
            for c in range(N_CHUNK):
                for q in range(n_q):
                    t = cpool.tile([P, quarter], F32R, tag=f"cb_{c}_{q}")
                    nc.sync.dma_start(out=t, in_=cb_tiles[c, q])
                    cb_sb[c, q] = t
            csq_sb = cpool.tile([2 * n_q, quarter], F32R, tag="csqpack")
            nc.sync.dma_start(out=csq_sb, in_=csqpack[:, :])
            sel_sb = {}
            for q in range(n_q):
                st = cpool.tile([2 * n_q, P], F32R, tag=f"sel_{q}")
                nc.sync.dma_start(out=st, in_=selrows[q])
                sel_sb[q] = st
            iota_sb = cpool.tile([P, quarter], F32, tag="iota")
            nc.sync.dma_start(out=iota_sb, in_=iota_b[:, :])
            iota_nq_sb = cpool.tile([P, n_q], F32, tag="iota_nq")
            nc.sync.dma_start(out=iota_nq_sb, in_=iota_nq[:, :])
            flags_sb = cpool.tile([P, n_tiles], F32, tag="flags")

            # --- main loop over token tiles ----------------------------------
            for tt in [t for _ in range(repeat) for t in range(n_tiles)]:
                xt = xpool.tile([P, N_CHUNK, P], F32R, tag="xt")
                nc.sync.dma_start(out=xt, in_=x_tiles[tt])

                qmax = smpool.tile([P, n_q], F32, tag="qmax")
                sq4 = smpool.tile([P, n_q], F32, tag="sq4")
                scores = []

                for q in range(n_q):
                    ps = pspool.tile([P, quarter], F32, tag="ps")
                    for c in range(N_CHUNK):
                        for s in range(n_sb):
                            nc.tensor.matmul(
                                out=ps[:, s * sb:(s + 1) * sb],
                                lhsT=xt[:, c, :],
                                rhs=cb_sb[c, q][:, s * sb:(s + 1) * sb],
                                start=(c == 0),
                                stop=False,
                            )
                    for s in range(n_sb):
                        nc.tensor.matmul(
                            out=ps[:, s * sb:(s + 1) * sb],
                            lhsT=sel_sb[q][:, :],
                            rhs=csq_sb[:, s * sb:(s + 1) * sb],
                            start=False,
                            stop=True,
                        )
                    # per-quarter max straight off PSUM (DVE)
                    nc.vector.reduce_max(
                        out=qmax[:, q:q + 1], in_=ps,
                        axis=mybir.AxisListType.X)
                    # drain scores PSUM -> SBUF on the idle ACT engine
                    score = spool.tile([P, quarter], F32, tag="score")
                    nc.scalar.copy(score, ps)
                    scores.append(score)

                # tau = gmax - DELTA
                gmax = sm1pool.tile([P, 1], F32, tag="gmax")
                nc.vector.reduce_max(
                    out=gmax, in_=qmax, axis=mybir.AxisListType.X)
                tau = sm1pool.tile([P, 1], F32, tag="tau")
                nc.vector.tensor_scalar_add(tau, gmax, -DELTA)

                # pass 2: S_q = sum_k (score_k >= tau) * (BPACK + k_local)
                # (all-SBUF operands -> DVE 2x_2p mode, 0.5 cyc/el)
                for q in range(n_q):
                    dummy = sm1pool.tile([P, 1], F32, tag=f"dummy{min(q, 1)}")
                    nc.vector.scalar_tensor_tensor(
                        out=dummy.broadcast_to((P, quarter)),
                        in0=scores[q],
                        scalar=tau,
                        in1=iota_sb,
                        op0=mybir.AluOpType.is_ge,
                        op1=mybir.AluOpType.mult,
                        accum_out=sq4[:, q:q + 1],
                    )

                # decode: count = floor(S/B) via f32->u32 convert (frac < 2^-6);
                # idx_local = S - count*B
                t1 = sm1pool.tile([P, n_q], F32, tag="t1")
                nc.vector.tensor_scalar_mul(t1, sq4, 1.0 / BPACK)
                cnt_u = sm1pool.tile([P, n_q], mybir.dt.uint32, tag="cnt_u")
                nc.vector.tensor_copy(cnt_u, t1)
                count4 = sm1pool.tile([P, n_q], F32, tag="count4")
                nc.vector.tensor_copy(count4, cnt_u)
                cntb = sm1pool.tile([P, n_q], F32, tag="cntb")
                nc.vector.tensor_scalar_mul(cntb, count4, BPACK)
                idx_local = sm1pool.tile([P, n_q], F32, tag="idx_local")
                nc.vector.tensor_sub(idx_local, sq4, cntb)
                nc.vector.reduce_sum(
                    out=flags_sb[:, tt:tt + 1], in_=count4,
                    axis=mybir.AxisListType.X)
                # global candidate index per quarter; select in-band quarters
                idxg = sm1pool.tile([P, n_q], F32, tag="idxg")
                nc.vector.tensor_add(idxg, idx_local, iota_nq_sb)
                idxf = sm1pool.tile([P, 1], F32, tag="idxf")
                dsel = sm1pool.tile([P, 1], F32, tag="dsel")
                nc.vector.scalar_tensor_tensor(
                    out=dsel.broadcast_to((P, n_q)),
                    in0=count4,
                    scalar=0.5,
                    in1=idxg,
                    op0=mybir.AluOpType.is_ge,
                    op1=mybir.AluOpType.mult,
                    accum_out=idxf,
                )
                idxc = sm1pool.tile([P, 1], F32, tag="idxc")
                nc.vector.tensor_scalar_min(idxc, idxf, float(k - 1))
                idxu = sm1pool.tile([P, 1], mybir.dt.uint32, tag="idxu")
                nc.vector.tensor_copy(idxu, idxc)

                # gather codebook rows and store
                rows = opool.tile([P, D], F32, tag="rows")
                nc.gpsimd.indirect_dma_start(
                    out=rows,
                    out_offset=None,
                    in_=codebook[:, :],
                    in_offset=bass.IndirectOffsetOnAxis(ap=idxu[:, :1], axis=0),
                )
                nc.sync.dma_start(out=out[tt * P:(tt + 1) * P, :], in_=rows)

            nc.sync.dma_start(out=out_flags[:, :], in_=flags_sb)

    return nc


def tf32_round(a, mant=MANT):
    """Round fp32 to `mant` explicit mantissa bits (round-to-nearest)."""
    ai = a.view(np.int32).astype(np.int64)
    shift = 23 - mant
    bias = 1 << (shift - 1)
    r = ((ai + bias) >> shift) << shift
    return r.astype(np.int32).view(np.float32)


def prep_core_inputs(x_core, shared, n_tiles):
    """Per-core input map. x_core: [n_tiles*P, D]."""
    xt = tf32_round(np.ascontiguousarray(
        x_core.reshape(n_tiles, P, N_CHUNK, P).transpose(0, 3, 2, 1)))
    return {"x_tiles": xt, **shared}


def prep_shared(codebook, k, quarter):
    n_q = k // quarter
    cb = np.ascontiguousarray(np.asarray(codebook, dtype=np.float32))
    cb2 = 2.0 * cb  # exact in fp32
    # cb_tiles[c, q, d, j] = cb2[q*quarter + j, c*128 + d]
    cb2_tiles = tf32_round(np.ascontiguousarray(
        cb2.reshape(n_q, quarter, N_CHUNK, P).transpose(2, 0, 3, 1)))
    csq = (cb * cb).sum(axis=1, dtype=np.float32)
    csqpack = np.zeros((2 * n_q, quarter), dtype=np.float32)
    selrows = np.zeros((n_q, 2 * n_q, P), dtype=np.float32)
    for q in range(n_q):
        seg = csq[q * quarter:(q + 1) * quarter]
        hi = tf32_round(-seg)
        lo = tf32_round(-seg - hi)
        csqpack[2 * q] = hi
        csqpack[2 * q + 1] = lo
        selrows[q, 2 * q, :] = 1.0
        selrows[q, 2 * q + 1, :] = 1.0
    iota_b_np = np.broadcast_to(
        (np.arange(quarter, dtype=np.float32) + BPACK)[None, :],
        (P, quarter)).copy()
    iota_nq_np = np.broadcast_to(
        (np.arange(n_q, dtype=np.float32) * quarter)[None, :], (P, n_q)).copy()
    return {
        "cb_tiles": cb2_tiles,
        "csqpack": csqpack,
        "selrows": selrows,
        "iota_b": iota_b_np,
        "iota_nq": iota_nq_np,
        "codebook": cb,
    }


_NC_CACHE = {}


def _get_nc(key):
    if key not in _NC_CACHE:
        nc = build_bass(*key)
        nc.finalize()
        _NC_CACHE[key] = nc
    return _NC_CACHE[key]


def _host_rescue(out_full, flags_full, x, codebook):
    """Recompute flagged tokens exactly (float64)."""
    bad = np.flatnonzero(flags_full != 1.0)
    if len(bad) == 0:
        return out_full, 0
    xb = x[bad].astype(np.float64)
    cb64 = codebook.astype(np.float64)
    csq = (cb64 * cb64).sum(1)
    sc = 2.0 * (xb @ cb64.T) - csq[None, :]
    idx = sc.argmax(1)
    out_full[bad] = codebook[idx]
    return out_full, len(bad)


def kernel(x, codebook):
    from concourse.bass_utils import run_bass_kernel_spmd

    x = np.ascontiguousarray(np.asarray(x, dtype=np.float32))
    codebook = np.ascontiguousarray(np.asarray(codebook, dtype=np.float32))
    assert x.shape == (N_TOKENS, D) and codebook.shape == (K, D)

    nc = _get_nc((N_TILES_FULL, K, QUARTER_FULL))
    shared = prep_shared(codebook, K, QUARTER_FULL)

    in_maps = []
    for core in range(N_CORES):
        x_core = x[core * T_PER_CORE:(core + 1) * T_PER_CORE]
        in_maps.append(prep_core_inputs(x_core, shared, N_TILES_FULL))

    res = run_bass_kernel_spmd(nc, in_maps, list(range(N_CORES)))
    out_full = np.concatenate(
        [res.results[i]["out"] for i in range(N_CORES)], axis=0)
    # flags: [P, n_tiles] per core; token (core, tt*128+p) at [p, tt]
    flags_full = np.concatenate(
        [np.asarray(res.results[i]["out_flags"]).T.reshape(-1)
         for i in range(N_CORES)])
    out_full, n_rescued = _host_rescue(out_full, flags_full, x, codebook)
    return out_full


# revision 15
# speedup vs baseline: 3.6545x; 1.6781x over previous
"""VQ codebook lookup (DiscreteDecisionEngine) on 8 TRN2 NeuronCores.

Math: for each token x_t, find argmin_k ||x_t - c_k||^2, emit codebook[k].
argmin_k ||x-c||^2 == argmax_k (2*x.c_k - ||c_k||^2)  (||x||^2 constant per token).

Device strategy (data-parallel over tokens, codebook replicated per core):
  - Token tile = 128 tokens. Scores for 8192 codes per tile computed as 4
    PSUM "quarters" of 2048 codes.
  - PE float32r (TF32-like, RNE to 11 mantissa bits, 1 cycle/row) matmuls:
    score = x @ (2C)^T - ||c||^2, with the ||c||^2 term folded in as a 5th
    K=2 contraction step (ones x [-csq_hi; -csq_lo] split keeps csq exact to
    ~3e-5 despite f32r rounding).
  - DVE reduce_max per quarter directly on PSUM -> qmax.
  - ACT drains PSUM -> SBUF score tiles (idle engine otherwise).
  - tau = global max - DELTA band. Pass 2 (scalar_tensor_tensor, DVE 2x_2p):
    S_q = sum_k (score >= tau) * (BPACK + k_local).
  - decode: count_q = S_q div BPACK; exactly one in-band code => exact index;
    total count emitted as a per-token flag.
  - GPSIMD indirect DMA gathers codebook rows, HWDGE stores output.

Host: tokens whose flag != 1 (a second code within DELTA of the max -- f32r
rounding could misrank those) are recomputed exactly in float64. Device score
error vs exact fp32 is bounded by ~0.07 (11-bit input rounding over D=512),
so DELTA=0.2 is sound with ~3x margin; ~1-2% of tokens get flagged.
"""

import numpy as np

import concourse.bacc as bacc
import concourse.bass as bass
import concourse.mybir as mybir
from concourse.tile import TileContext

P = 128          # partitions / token tile
D = 512          # latent dim
K = 8192         # codebook size
N_TOKENS = 32768
N_CORES = 8
T_PER_CORE = N_TOKENS // N_CORES   # 4096
N_TILES_FULL = T_PER_CORE // P     # 32
QUARTER_FULL = 2048                # codes per PSUM quarter (4 banks)
N_CHUNK = D // P                   # 4 contraction chunks
MM_N = 512                         # matmul free-dim block (1 PSUM bank, fp32)

F32 = mybir.dt.float32
F32R = mybir.dt.float32r
BPACK = float(1 << 17)             # count-packing base in pass 2
DELTA = 0.2                        # at-risk band below the device max
MANT = 11                          # f32r = RNE to 11 explicit mantissa bits


def build_bass(n_tiles=N_TILES_FULL, k=K, quarter=QUARTER_FULL, repeat=1):
    """Build the single-core Bass program (SPMD across cores)."""
    n_q = k // quarter
    n_sb = max(1, quarter // MM_N)
    sb = min(MM_N, quarter)

    nc = bacc.Bacc()
    x_tiles = nc.declare_dram_parameter(
        "x_tiles", [n_tiles, P, N_CHUNK, P], F32R, isOutput=False)
    cb_tiles = nc.declare_dram_parameter(
        "cb_tiles", [N_CHUNK, n_q, P, quarter], F32R, isOutput=False)
    # rows 2q / 2q+1 hold -csq_hi / -csq_lo for quarter q; the per-quarter
    # K=8 selector weight (0/1 rows) picks the right pair so every matmul
    # anchors at base partition 0.
    csqpack = nc.declare_dram_parameter("csqpack", [2 * n_q, quarter], F32R,
                                        isOutput=False)
    selrows = nc.declare_dram_parameter("selrows", [n_q, 2 * n_q, P], F32R,
                                        isOutput=False)
    iota_b = nc.declare_dram_parameter("iota_b", [P, quarter], F32,
                                       isOutput=False)
    iota_nq = nc.declare_dram_parameter(
        "iota_nq", [P, 8 * n_q], F32, isOutput=False)
    codebook = nc.declare_dram_parameter("codebook", [k, D], F32,
                                         isOutput=False)
    out = nc.declare_dram_parameter("out", [n_tiles * P, D], F32,
                                    isOutput=True)
    out_flags = nc.declare_dram_parameter(
        "out_flags", [P, n_tiles], F32, isOutput=True)

    with TileContext(nc) as tc:
        with (
            tc.tile_pool(name="const", bufs=1) as cpool,
            tc.tile_pool(name="xp", bufs=3) as xpool,
            tc.tile_pool(name="sp", bufs=5) as spool,
            tc.tile_pool(name="small", bufs=2) as smpool,
            tc.tile_pool(name="sm1", bufs=1) as sm1pool,
            tc.tile_pool(name="op", bufs=2) as opool,
            tc.tile_pool(name="ps", bufs=2, space="PSUM") as pspool,
        ):
            # --- resident constants ------------------------------------------
            # small consts first (sync queue), then codebook tiles spread
            # across the three DMA issuers so the first quarter lands fast
            csq_sb = cpool.tile([2 * n_q, quarter], F32R, tag="csqpack")
            nc.sync.dma_start(out=csq_sb, in_=csqpack[:, :])
            sel_sb = {}
            for q in range(n_q):
                st = cpool.tile([2 * n_q, P], F32R, tag=f"sel_{q}")
                nc.sync.dma_start(out=st, in_=selrows[q])
                sel_sb[q] = st
            iota_sb = cpool.tile([P, quarter], F32, tag="iota")
            nc.scalar.dma_start(out=iota_sb, in_=iota_b[:, :])
            iota_nq_sb = cpool.tile([P, 8 * n_q], F32, tag="iota_nq")
            nc.scalar.dma_start(out=iota_nq_sb, in_=iota_nq[:, :])
            flags_sb = cpool.tile([P, n_tiles], F32, tag="flags")
            cb_sb = {}
            dma_engs = [nc.sync, nc.scalar, nc.gpsimd]
            for j, (q, c) in enumerate(
                    (q, c) for q in range(n_q) for c in range(N_CHUNK)):
                t = cpool.tile([P, quarter], F32R, tag=f"cb_{c}_{q}")
                dma_engs[j % 3].dma_start(out=t, in_=cb_tiles[c, q])
                cb_sb[c, q] = t

            # --- main loop over token tiles ----------------------------------
            BATCH = 8
            assert n_tiles % BATCH == 0 or n_tiles < BATCH
            batch = min(BATCH, n_tiles)
            tts = [t for _ in range(repeat) for t in range(n_tiles)]
            for bstart in range(0, len(tts), batch):
                btiles = tts[bstart:bstart + batch]
                nb = len(btiles)
                s_all = smpool.tile([P, nb, n_q], F32, tag="s_all")
                for bi, tt in enumerate(btiles):
                    xt = xpool.tile([P, N_CHUNK, P], F32R, tag="xt")
                    nc.sync.dma_start(out=xt, in_=x_tiles[tt])

                    qmax = smpool.tile([P, n_q], F32, tag="qmax")
                    scores = []
                    for q in range(n_q):
                        ps = pspool.tile([P, quarter], F32, tag="ps")
                        for c in range(N_CHUNK):
                            for s in range(n_sb):
                                nc.tensor.matmul(
                                    out=ps[:, s * sb:(s + 1) * sb],
                                    lhsT=xt[:, c, :],
                                    rhs=cb_sb[c, q][:, s * sb:(s + 1) * sb],
                                    start=(c == 0),
                                    stop=False,
                                )
                        for s in range(n_sb):
                            nc.tensor.matmul(
                                out=ps[:, s * sb:(s + 1) * sb],
                                lhsT=sel_sb[q][:, :],
                                rhs=csq_sb[:, s * sb:(s + 1) * sb],
                                start=False,
                                stop=True,
                            )
                        # per-quarter max straight off PSUM (DVE)
                        nc.vector.reduce_max(
                            out=qmax[:, q:q + 1], in_=ps,
                            axis=mybir.AxisListType.X)
                        # drain scores PSUM -> SBUF on the idle ACT engine
                        score = spool.tile([P, quarter], F32, tag="score")
                        nc.scalar.copy(score, ps)
                        scores.append(score)

                    # tau = gmax - DELTA
                    gmax = sm1pool.tile([P, 1], F32, tag="gmax")
                    nc.vector.reduce_max(
                        out=gmax, in_=qmax, axis=mybir.AxisListType.X)
                    tau = sm1pool.tile([P, 1], F32, tag="tau")
                    nc.vector.tensor_scalar_add(tau, gmax, -DELTA)

                    # pass 2: S_q = sum_k (score_k >= tau) * (BPACK + k_local)
                    for q in range(n_q):
                        dummy = sm1pool.tile(
                            [P, 1], F32, tag=f"dummy{min(q, 1)}")
                        nc.vector.scalar_tensor_tensor(
                            out=dummy.broadcast_to((P, quarter)),
                            in0=scores[q],
                            scalar=tau,
                            in1=iota_sb,
                            op0=mybir.AluOpType.is_ge,
                            op1=mybir.AluOpType.mult,
                            accum_out=s_all[:, bi, q:q + 1],
                        )

                # ---- batched decode over `nb` tiles ([P, nb*n_q] ops) -------
                nbq = nb * n_q
                sflat = s_all.rearrange("p b q -> p (b q)")
                t1 = sm1pool.tile([P, nbq], F32, tag="t1")
                nc.vector.tensor_scalar_mul(t1, sflat, 1.0 / BPACK)
                cnt_u = sm1pool.tile([P, nbq], mybir.dt.uint32, tag="cnt_u")
                nc.vector.tensor_copy(cnt_u, t1)
                countb = sm1pool.tile([P, nbq], F32, tag="countb")
                nc.vector.tensor_copy(countb, cnt_u)
                cntb = sm1pool.tile([P, nbq], F32, tag="cntb")
                nc.vector.tensor_scalar_mul(cntb, countb, BPACK)
                idx_local = sm1pool.tile([P, nbq], F32, tag="idx_local")
                nc.vector.tensor_sub(idx_local, sflat, cntb)
                # per-tile flags: sum counts over the quarter axis
                nc.vector.reduce_sum(
                    out=flags_sb[:, bstart % n_tiles:
                                 bstart % n_tiles + nb],
                    in_=countb.rearrange("p (b q) -> p b q", q=n_q),
                    axis=mybir.AxisListType.X)
                # global candidate index; select in-band quarters
                idxg = sm1pool.tile([P, nbq], F32, tag="idxg")
                nc.vector.tensor_add(idxg, idx_local, iota_nq_sb[:, :nbq])
                masked = sm1pool.tile([P, nbq], F32, tag="masked")
                nc.vector.scalar_tensor_tensor(
                    out=masked,
                    in0=countb,
                    scalar=0.5,
                    in1=idxg,
                    op0=mybir.AluOpType.is_ge,
                    op1=mybir.AluOpType.mult,
                )
                idxf = sm1pool.tile([P, nb], F32, tag="idxf")
                nc.vector.reduce_sum(
                    out=idxf,
                    in_=masked.rearrange("p (b q) -> p b q", q=n_q),
                    axis=mybir.AxisListType.X)
                idxc = sm1pool.tile([P, nb], F32, tag="idxc")
                nc.vector.tensor_scalar_min(idxc, idxf, float(k - 1))
                idxu = sm1pool.tile([P, nb], mybir.dt.uint32, tag="idxu")
                nc.vector.tensor_copy(idxu, idxc)

                # gather codebook rows and store, per tile in the batch
                for bi, tt in enumerate(btiles):
                    rows = opool.tile([P, D], F32, tag="rows")
                    nc.gpsimd.indirect_dma_start(
                        out=rows,
                        out_offset=None,
                        in_=codebook[:, :],
                        in_offset=bass.IndirectOffsetOnAxis(
                            ap=idxu[:, bi:bi + 1], axis=0),
                    )
                    nc.sync.dma_start(
                        out=out[tt * P:(tt + 1) * P, :], in_=rows)

            nc.sync.dma_start(out=out_flags[:, :], in_=flags_sb)

    return nc


def tf32_round(a, mant=MANT):
    """Round fp32 to `mant` explicit mantissa bits (round-to-nearest)."""
    ai = a.view(np.int32).astype(np.int64)
    shift = 23 - mant
    bias = 1 << (shift - 1)
    r = ((ai + bias) >> shift) << shift
    return r.astype(np.int32).view(np.float32)


def prep_core_inputs(x_core, shared, n_tiles):
    """Per-core input map. x_core: [n_tiles*P, D]."""
    xt = tf32_round(np.ascontiguousarray(
        x_core.reshape(n_tiles, P, N_CHUNK, P).transpose(0, 3, 2, 1)))
    return {"x_tiles": xt, **shared}


def prep_shared(codebook, k, quarter):
    n_q = k // quarter
    cb = np.ascontiguousarray(np.asarray(codebook, dtype=np.float32))
    cb2 = 2.0 * cb  # exact in fp32
    # cb_tiles[c, q, d, j] = cb2[q*quarter + j, c*128 + d]
    cb2_tiles = tf32_round(np.ascontiguousarray(
        cb2.reshape(n_q, quarter, N_CHUNK, P).transpose(2, 0, 3, 1)))
    csq = (cb * cb).sum(axis=1, dtype=np.float32)
    csqpack = np.zeros((2 * n_q, quarter), dtype=np.float32)
    selrows = np.zeros((n_q, 2 * n_q, P), dtype=np.float32)
    for q in range(n_q):
        seg = csq[q * quarter:(q + 1) * quarter]
        hi = tf32_round(-seg)
        lo = tf32_round(-seg - hi)
        csqpack[2 * q] = hi
        csqpack[2 * q + 1] = lo
        selrows[q, 2 * q, :] = 1.0
        selrows[q, 2 * q + 1, :] = 1.0
    iota_b_np = np.broadcast_to(
        (np.arange(quarter, dtype=np.float32) + BPACK)[None, :],
        (P, quarter)).copy()
    offs = np.tile(np.arange(n_q, dtype=np.float32) * quarter, 8)
    iota_nq_np = np.broadcast_to(offs[None, :], (P, len(offs))).copy()
    return {
        "cb_tiles": cb2_tiles,
        "csqpack": csqpack,
        "selrows": selrows,
        "iota_b": iota_b_np,
        "iota_nq": iota_nq_np,
        "codebook": cb,
    }


_NC_CACHE = {}


def _get_nc(key):
    if key not in _NC_CACHE:
        nc = build_bass(*key)
        nc.finalize()
        _NC_CACHE[key] = nc
    return _NC_CACHE[key]


def _host_rescue(out_full, flags_full, x, codebook):
    """Recompute flagged tokens exactly (float64)."""
    bad = np.flatnonzero(flags_full != 1.0)
    if len(bad) == 0:
        return out_full, 0
    xb = x[bad].astype(np.float64)
    cb64 = codebook.astype(np.float64)
    csq = (cb64 * cb64).sum(1)
    sc = 2.0 * (xb @ cb64.T) - csq[None, :]
    idx = sc.argmax(1)
    out_full[bad] = codebook[idx]
    return out_full, len(bad)


def kernel(x, codebook):
    from concourse.bass_utils import run_bass_kernel_spmd

    x = np.ascontiguousarray(np.asarray(x, dtype=np.float32))
    codebook = np.ascontiguousarray(np.asarray(codebook, dtype=np.float32))
    assert x.shape == (N_TOKENS, D) and codebook.shape == (K, D)

    nc = _get_nc((N_TILES_FULL, K, QUARTER_FULL))
    shared = prep_shared(codebook, K, QUARTER_FULL)

    in_maps = []
    for core in range(N_CORES):
        x_core = x[core * T_PER_CORE:(core + 1) * T_PER_CORE]
        in_maps.append(prep_core_inputs(x_core, shared, N_TILES_FULL))

    res = run_bass_kernel_spmd(nc, in_maps, list(range(N_CORES)))
    out_full = np.concatenate(
        [res.results[i]["out"] for i in range(N_CORES)], axis=0)
    # flags: [P, n_tiles] per core; token (core, tt*128+p) at [p, tt]
    flags_full = np.concatenate(
        [np.asarray(res.results[i]["out_flags"]).T.reshape(-1)
         for i in range(N_CORES)])
    out_full, n_rescued = _host_rescue(out_full, flags_full, x, codebook)
    return out_full


# revision 20
# speedup vs baseline: 4.4010x; 1.2043x over previous
"""VQ codebook lookup (DiscreteDecisionEngine) on 8 TRN2 NeuronCores.

Math: for each token x_t, find argmin_k ||x_t - c_k||^2, emit codebook[k].
argmin_k ||x-c||^2 == argmax_k (2*x.c_k - ||c_k||^2)  (||x||^2 constant per token).

Device strategy (data-parallel over tokens, codebook replicated per core):
  - Token tile = 128 tokens. Scores for 8192 codes per tile computed as 4
    PSUM "quarters" of 2048 codes.
  - PE float32r (TF32-like, RNE to 11 mantissa bits, 1 cycle/row) matmuls:
    score = x @ (2C)^T - ||c||^2, with the ||c||^2 term folded in as a 5th
    K=2 contraction step (ones x [-csq_hi; -csq_lo] split keeps csq exact to
    ~3e-5 despite f32r rounding).
  - DVE reduce_max per quarter directly on PSUM -> qmax.
  - ACT drains PSUM -> SBUF score tiles (idle engine otherwise).
  - tau = global max - DELTA band. Pass 2 (scalar_tensor_tensor, DVE 2x_2p):
    S_q = sum_k (score >= tau) * (BPACK + k_local).
  - decode: count_q = S_q div BPACK; exactly one in-band code => exact index;
    total count emitted as a per-token flag.
  - GPSIMD indirect DMA gathers codebook rows, HWDGE stores output.

Host: tokens whose flag != 1 (a second code within DELTA of the max -- f32r
rounding could misrank those) are recomputed exactly in float64. Device score
error vs exact fp32 is bounded by ~0.07 (11-bit input rounding over D=512),
so DELTA=0.2 is sound with ~3x margin; ~1-2% of tokens get flagged.
"""

import numpy as np

import concourse.bacc as bacc
import concourse.bass as bass
import concourse.mybir as mybir
from concourse.tile import TileContext

P = 128          # partitions / token tile
D = 512          # latent dim
K = 8192         # codebook size
N_TOKENS = 32768
N_CORES = 8
T_PER_CORE = N_TOKENS // N_CORES   # 4096
N_TILES_FULL = T_PER_CORE // P     # 32
QUARTER_FULL = 2048                # codes per PSUM quarter (4 banks)
N_CHUNK = D // P                   # 4 contraction chunks
MM_N = 512                         # matmul free-dim block (1 PSUM bank, fp32)

F32 = mybir.dt.float32
F32R = mybir.dt.float32r
BPACK = float(1 << 17)             # count-packing base in pass 2
DELTA = 0.2                        # at-risk band below the device max
MANT = 11                          # f32r = RNE to 11 explicit mantissa bits


def build_bass(n_tiles=N_TILES_FULL, k=K, quarter=QUARTER_FULL, repeat=1):
    """Build the single-core Bass program (SPMD across cores)."""
    n_q = k // quarter
    n_sb = max(1, quarter // MM_N)
    sb = min(MM_N, quarter)

    nc = bacc.Bacc()
    x_tiles = nc.declare_dram_parameter(
        "x_tiles", [n_tiles, P, N_CHUNK, P], F32R, isOutput=False)
    cb_tiles = nc.declare_dram_parameter(
        "cb_tiles", [N_CHUNK, n_q, P, quarter], F32R, isOutput=False)
    # rows 2q / 2q+1 hold -csq_hi / -csq_lo for quarter q; the per-quarter
    # K=8 selector weight (0/1 rows) picks the right pair so every matmul
    # anchors at base partition 0.
    csqpack = nc.declare_dram_parameter("csqpack", [2 * n_q, quarter], F32R,
                                        isOutput=False)
    selrows = nc.declare_dram_parameter("selrows", [n_q, 2 * n_q, P], F32R,
                                        isOutput=False)
    iota_b = nc.declare_dram_parameter("iota_b", [P, quarter], F32,
                                       isOutput=False)
    iota_nq = nc.declare_dram_parameter(
        "iota_nq", [P, 8 * n_q], F32, isOutput=False)
    codebook = nc.declare_dram_parameter("codebook", [k, D], F32,
                                         isOutput=False)
    out = nc.declare_dram_parameter("out", [n_tiles * P, D], F32,
                                    isOutput=True)
    out_flags = nc.declare_dram_parameter(
        "out_flags", [P, n_tiles], F32, isOutput=True)

    with TileContext(nc) as tc:
        with (
            tc.tile_pool(name="const", bufs=1) as cpool,
            tc.tile_pool(name="xp", bufs=3) as xpool,
            tc.tile_pool(name="sp", bufs=5) as spool,
            tc.tile_pool(name="small", bufs=2) as smpool,
            tc.tile_pool(name="sm1", bufs=1) as sm1pool,
            tc.tile_pool(name="op", bufs=2) as opool,
            tc.tile_pool(name="ps", bufs=2, space="PSUM") as pspool,
        ):
            # --- resident constants ------------------------------------------
            # small consts first (sync queue), then codebook tiles spread
            # across the three DMA issuers so the first quarter lands fast
            csq_sb = cpool.tile([2 * n_q, quarter], F32R, tag="csqpack")
            nc.sync.dma_start(out=csq_sb, in_=csqpack[:, :])
            sel_sb = {}
            for q in range(n_q):
                st = cpool.tile([2 * n_q, P], F32R, tag=f"sel_{q}")
                nc.sync.dma_start(out=st, in_=selrows[q])
                sel_sb[q] = st
            iota_sb = cpool.tile([P, quarter], F32, tag="iota")
            nc.scalar.dma_start(out=iota_sb, in_=iota_b[:, :])
            iota_nq_sb = cpool.tile([P, 8 * n_q], F32, tag="iota_nq")
            nc.scalar.dma_start(out=iota_nq_sb, in_=iota_nq[:, :])
            flags_sb = cpool.tile([P, n_tiles], F32, tag="flags")
            cb_sb = {}
            dma_engs = [nc.sync, nc.scalar, nc.gpsimd]
            for j, (q, c) in enumerate(
                    (q, c) for q in range(n_q) for c in range(N_CHUNK)):
                t = cpool.tile([P, quarter], F32R, tag=f"cb_{c}_{q}")
                dma_engs[j % 3].dma_start(out=t, in_=cb_tiles[c, q])
                cb_sb[c, q] = t

            # --- main loop over token tiles ----------------------------------
            BATCH = 8
            assert n_tiles % BATCH == 0 or n_tiles < BATCH
            batch = min(BATCH, n_tiles)
            tts = [t for _ in range(repeat) for t in range(n_tiles)]
            for bstart in range(0, len(tts), batch):
                btiles = tts[bstart:bstart + batch]
                nb = len(btiles)
                s_all = smpool.tile([P, nb, n_q], F32, tag="s_all")
                for bi, tt in enumerate(btiles):
                    xt = xpool.tile([P, N_CHUNK, P], F32R, tag="xt")
                    nc.sync.dma_start(out=xt, in_=x_tiles[tt])

                    qmax = smpool.tile([P, n_q], F32, tag="qmax")
                    scores = []
                    for q in range(n_q):
                        ps = pspool.tile([P, quarter], F32, tag="ps")
                        for c in range(N_CHUNK):
                            for s in range(n_sb):
                                nc.tensor.matmul(
                                    out=ps[:, s * sb:(s + 1) * sb],
                                    lhsT=xt[:, c, :],
                                    rhs=cb_sb[c, q][:, s * sb:(s + 1) * sb],
                                    start=(c == 0),
                                    stop=False,
                                )
                        for s in range(n_sb):
                            nc.tensor.matmul(
                                out=ps[:, s * sb:(s + 1) * sb],
                                lhsT=sel_sb[q][:, :],
                                rhs=csq_sb[:, s * sb:(s + 1) * sb],
                                start=False,
                                stop=True,
                            )
                        # drain scores PSUM -> SBUF on the idle ACT engine;
                        # DVE reduces the SBUF copy so the PSUM bank is
                        # released by ACT alone (DVE runs backlogged and
                        # would stall PE's next accumulation group)
                        score = spool.tile([P, quarter], F32, tag="score")
                        nc.scalar.copy(score, ps)
                        nc.vector.reduce_max(
                            out=qmax[:, q:q + 1], in_=score,
                            axis=mybir.AxisListType.X)
                        scores.append(score)

                    # tau = gmax - DELTA
                    gmax = sm1pool.tile([P, 1], F32, tag="gmax")
                    nc.vector.reduce_max(
                        out=gmax, in_=qmax, axis=mybir.AxisListType.X)
                    tau = sm1pool.tile([P, 1], F32, tag="tau")
                    nc.vector.tensor_scalar_add(tau, gmax, -DELTA)

                    # pass 2: S_q = sum_k (score_k >= tau) * (BPACK + k_local)
                    for q in range(n_q):
                        dummy = sm1pool.tile(
                            [P, 1], F32, tag=f"dummy{min(q, 1)}")
                        nc.vector.scalar_tensor_tensor(
                            out=dummy.broadcast_to((P, quarter)),
                            in0=scores[q],
                            scalar=tau,
                            in1=iota_sb,
                            op0=mybir.AluOpType.is_ge,
                            op1=mybir.AluOpType.mult,
                            accum_out=s_all[:, bi, q:q + 1],
                        )

                # ---- batched decode over `nb` tiles ([P, nb*n_q] ops) -------
                nbq = nb * n_q
                sflat = s_all.rearrange("p b q -> p (b q)")
                t1 = sm1pool.tile([P, nbq], F32, tag="t1")
                nc.vector.tensor_scalar_mul(t1, sflat, 1.0 / BPACK)
                cnt_u = sm1pool.tile([P, nbq], mybir.dt.uint32, tag="cnt_u")
                nc.vector.tensor_copy(cnt_u, t1)
                countb = sm1pool.tile([P, nbq], F32, tag="countb")
                nc.vector.tensor_copy(countb, cnt_u)
                cntb = sm1pool.tile([P, nbq], F32, tag="cntb")
                nc.vector.tensor_scalar_mul(cntb, countb, BPACK)
                idx_local = sm1pool.tile([P, nbq], F32, tag="idx_local")
                nc.vector.tensor_sub(idx_local, sflat, cntb)
                # per-tile flags: sum counts over the quarter axis
                nc.vector.reduce_sum(
                    out=flags_sb[:, bstart % n_tiles:
                                 bstart % n_tiles + nb],
                    in_=countb.rearrange("p (b q) -> p b q", q=n_q),
                    axis=mybir.AxisListType.X)
                # global candidate index; select in-band quarters
                idxg = sm1pool.tile([P, nbq], F32, tag="idxg")
                nc.vector.tensor_add(idxg, idx_local, iota_nq_sb[:, :nbq])
                masked = sm1pool.tile([P, nbq], F32, tag="masked")
                nc.vector.scalar_tensor_tensor(
                    out=masked,
                    in0=countb,
                    scalar=0.5,
                    in1=idxg,
                    op0=mybir.AluOpType.is_ge,
                    op1=mybir.AluOpType.mult,
                )
                idxf = sm1pool.tile([P, nb], F32, tag="idxf")
                nc.vector.reduce_sum(
                    out=idxf,
                    in_=masked.rearrange("p (b q) -> p b q", q=n_q),
                    axis=mybir.AxisListType.X)
                idxc = sm1pool.tile([P, nb], F32, tag="idxc")
                nc.vector.tensor_scalar_min(idxc, idxf, float(k - 1))
                idxu = sm1pool.tile([P, nb], mybir.dt.uint32, tag="idxu")
                nc.vector.tensor_copy(idxu, idxc)

                # gather codebook rows and store, per tile in the batch
                for bi, tt in enumerate(btiles):
                    rows = opool.tile([P, D], F32, tag="rows")
                    nc.gpsimd.indirect_dma_start(
                        out=rows,
                        out_offset=None,
                        in_=codebook[:, :],
                        in_offset=bass.IndirectOffsetOnAxis(
                            ap=idxu[:, bi:bi + 1], axis=0),
                    )
                    nc.sync.dma_start(
                        out=out[tt * P:(tt + 1) * P, :], in_=rows)

            nc.sync.dma_start(out=out_flags[:, :], in_=flags_sb)

    return nc


def tf32_round(a, mant=MANT):
    """Round fp32 to `mant` explicit mantissa bits (round-to-nearest)."""
    ai = a.view(np.int32).astype(np.int64)
    shift = 23 - mant
    bias = 1 << (shift - 1)
    r = ((ai + bias) >> shift) << shift
    return r.astype(np.int32).view(np.float32)


def prep_core_inputs(x_core, shared, n_tiles):
    """Per-core input map. x_core: [n_tiles*P, D]."""
    xt = tf32_round(np.ascontiguousarray(
        x_core.reshape(n_tiles, P, N_CHUNK, P).transpose(0, 3, 2, 1)))
    return {"x_tiles": xt, **shared}


def prep_shared(codebook, k, quarter):
    n_q = k // quarter
    cb = np.ascontiguousarray(np.asarray(codebook, dtype=np.float32))
    cb2 = 2.0 * cb  # exact in fp32
    # cb_tiles[c, q, d, j] = cb2[q*quarter + j, c*128 + d]
    cb2_tiles = tf32_round(np.ascontiguousarray(
        cb2.reshape(n_q, quarter, N_CHUNK, P).transpose(2, 0, 3, 1)))
    csq = (cb * cb).sum(axis=1, dtype=np.float32)
    csqpack = np.zeros((2 * n_q, quarter), dtype=np.float32)
    selrows = np.zeros((n_q, 2 * n_q, P), dtype=np.float32)
    for q in range(n_q):
        seg = csq[q * quarter:(q + 1) * quarter]
        hi = tf32_round(-seg)
        lo = tf32_round(-seg - hi)
        csqpack[2 * q] = hi
        csqpack[2 * q + 1] = lo
        selrows[q, 2 * q, :] = 1.0
        selrows[q, 2 * q + 1, :] = 1.0
    iota_b_np = np.broadcast_to(
        (np.arange(quarter, dtype=np.float32) + BPACK)[None, :],
        (P, quarter)).copy()
    offs = np.tile(np.arange(n_q, dtype=np.float32) * quarter, 8)
    iota_nq_np = np.broadcast_to(offs[None, :], (P, len(offs))).copy()
    return {
        "cb_tiles": cb2_tiles,
        "csqpack": csqpack,
        "selrows": selrows,
        "iota_b": iota_b_np,
        "iota_nq": iota_nq_np,
        "codebook": cb,
    }


_NC_CACHE = {}


def _get_nc(key):
    if key not in _NC_CACHE:
        nc = build_bass(*key)
        nc.finalize()
        _NC_CACHE[key] = nc
    return _NC_CACHE[key]


def _host_rescue(out_full, flags_full, x, codebook):
    """Recompute flagged tokens exactly (float64)."""
    bad = np.flatnonzero(flags_full != 1.0)
    if len(bad) == 0:
        return out_full, 0
    xb = x[bad].astype(np.float64)
    cb64 = codebook.astype(np.float64)
    csq = (cb64 * cb64).sum(1)
    sc = 2.0 * (xb @ cb64.T) - csq[None, :]
    idx = sc.argmax(1)
    out_full[bad] = codebook[idx]
    return out_full, len(bad)


def kernel(x, codebook):
    from concourse.bass_utils import run_bass_kernel_spmd

    x = np.ascontiguousarray(np.asarray(x, dtype=np.float32))
    codebook = np.ascontiguousarray(np.asarray(codebook, dtype=np.float32))
    assert x.shape == (N_TOKENS, D) and codebook.shape == (K, D)

    nc = _get_nc((N_TILES_FULL, K, QUARTER_FULL))
    shared = prep_shared(codebook, K, QUARTER_FULL)

    in_maps = []
    for core in range(N_CORES):
        x_core = x[core * T_PER_CORE:(core + 1) * T_PER_CORE]
        in_maps.append(prep_core_inputs(x_core, shared, N_TILES_FULL))

    res = run_bass_kernel_spmd(nc, in_maps, list(range(N_CORES)))
    out_full = np.concatenate(
        [res.results[i]["out"] for i in range(N_CORES)], axis=0)
    # flags: [P, n_tiles] per core; token (core, tt*128+p) at [p, tt]
    flags_full = np.concatenate(
        [np.asarray(res.results[i]["out_flags"]).T.reshape(-1)
         for i in range(N_CORES)])
    out_full, n_rescued = _host_rescue(out_full, flags_full, x, codebook)
    return out_full


# revision 21
# speedup vs baseline: 21.7301x; 4.9375x over previous
"""VQ codebook lookup (DiscreteDecisionEngine) on 8 TRN2 NeuronCores.

Math: for each token x_t, find argmin_k ||x_t - c_k||^2, emit codebook[k].
argmin_k ||x-c||^2 == argmax_k (2*x.c_k - ||c_k||^2)  (||x||^2 constant per token).

Device strategy (data-parallel over tokens, codebook replicated per core):
  - Token tile = 128 tokens. Scores for 8192 codes per tile computed as 4
    PSUM "quarters" of 2048 codes.
  - PE float32r (TF32-like, RNE to 11 mantissa bits, 1 cycle/row) matmuls:
    score = x @ (2C)^T - ||c||^2, with the ||c||^2 term folded in as a 5th
    K=2 contraction step (ones x [-csq_hi; -csq_lo] split keeps csq exact to
    ~3e-5 despite f32r rounding).
  - DVE reduce_max per quarter directly on PSUM -> qmax.
  - ACT drains PSUM -> SBUF score tiles (idle engine otherwise).
  - tau = global max - DELTA band. Pass 2 (scalar_tensor_tensor, DVE 2x_2p):
    S_q = sum_k (score >= tau) * (BPACK + k_local).
  - decode: count_q = S_q div BPACK; exactly one in-band code => exact index;
    total count emitted as a per-token flag.
  - GPSIMD indirect DMA gathers codebook rows, HWDGE stores output.

Host: tokens whose flag != 1 (a second code within DELTA of the max -- f32r
rounding could misrank those) are recomputed exactly in float64. Device score
error vs exact fp32 is bounded by ~0.07 (11-bit input rounding over D=512),
so DELTA=0.2 is sound with ~3x margin; ~1-2% of tokens get flagged.
"""

import numpy as np

import concourse.bacc as bacc
import concourse.bass as bass
import concourse.mybir as mybir
from concourse.tile import TileContext

P = 128          # partitions / token tile
D = 512          # latent dim
K = 8192         # codebook size
N_TOKENS = 32768
N_CORES = 8
T_PER_CORE = N_TOKENS // N_CORES   # 4096
N_TILES_FULL = T_PER_CORE // P     # 32
QUARTER_FULL = 2048                # codes per PSUM quarter (4 banks)
N_CHUNK = D // P                   # 4 contraction chunks
MM_N = 512                         # matmul free-dim block (1 PSUM bank, fp32)

F32 = mybir.dt.float32
F32R = mybir.dt.float32r
BPACK = float(1 << 17)             # count-packing base in pass 2
DELTA = 0.2                        # at-risk band below the device max
MANT = 11                          # f32r = RNE to 11 explicit mantissa bits


def build_bass(n_tiles=N_TILES_FULL, k=K, quarter=QUARTER_FULL, repeat=1):
    """Build the single-core Bass program (SPMD across cores)."""
    n_q = k // quarter
    n_sb = max(1, quarter // MM_N)
    sb = min(MM_N, quarter)

    nc = bacc.Bacc()
    x_tiles = nc.declare_dram_parameter(
        "x_tiles", [n_tiles, P, N_CHUNK, P], F32R, isOutput=False)
    cb_tiles = nc.declare_dram_parameter(
        "cb_tiles", [N_CHUNK, n_q, P, quarter], F32R, isOutput=False)
    # rows 2q / 2q+1 hold -csq_hi / -csq_lo for quarter q; the per-quarter
    # K=8 selector weight (0/1 rows) picks the right pair so every matmul
    # anchors at base partition 0.
    csqpack = nc.declare_dram_parameter("csqpack", [2 * n_q, quarter], F32R,
                                        isOutput=False)
    selrows = nc.declare_dram_parameter("selrows", [n_q, 2 * n_q, P], F32R,
                                        isOutput=False)
    iota_b = nc.declare_dram_parameter("iota_b", [P, quarter], F32,
                                       isOutput=False)
    iota_nq = nc.declare_dram_parameter(
        "iota_nq", [P, 8 * n_q], F32, isOutput=False)
    codebook = nc.declare_dram_parameter("codebook", [k, D], F32,
                                         isOutput=False)
    out = nc.declare_dram_parameter("out", [n_tiles * P, D], F32,
                                    isOutput=True)
    out_flags = nc.declare_dram_parameter(
        "out_flags", [P, n_tiles], F32, isOutput=True)

    with TileContext(nc) as tc:
        with (
            tc.tile_pool(name="const", bufs=1) as cpool,
            tc.tile_pool(name="xp", bufs=3) as xpool,
            tc.tile_pool(name="sp", bufs=5) as spool,
            tc.tile_pool(name="small", bufs=2) as smpool,
            tc.tile_pool(name="sm1", bufs=1) as sm1pool,
            tc.tile_pool(name="op", bufs=2) as opool,
            tc.tile_pool(name="ps", bufs=2, space="PSUM") as pspool,
        ):
            # --- resident constants ------------------------------------------
            # small consts first (sync queue), then codebook tiles spread
            # across the three DMA issuers so the first quarter lands fast
            csq_sb = cpool.tile([2 * n_q, quarter], F32R, tag="csqpack")
            nc.sync.dma_start(out=csq_sb, in_=csqpack[:, :])
            sel_sb = {}
            for q in range(n_q):
                st = cpool.tile([2 * n_q, P], F32R, tag=f"sel_{q}")
                nc.sync.dma_start(out=st, in_=selrows[q])
                sel_sb[q] = st
            iota_sb = cpool.tile([P, quarter], F32, tag="iota")
            nc.scalar.dma_start(out=iota_sb, in_=iota_b[:, :])
            iota_nq_sb = cpool.tile([P, 8 * n_q], F32, tag="iota_nq")
            nc.scalar.dma_start(out=iota_nq_sb, in_=iota_nq[:, :])
            flags_sb = cpool.tile([P, n_tiles], F32, tag="flags")
            # fine-grained [P, sb] codebook tiles: the first matmul only
            # waits on a 256KB transfer instead of a full 1MB quarter-chunk
            cb_sb = {}
            dma_engs = [nc.sync, nc.scalar, nc.gpsimd]
            for j, (q, s, c) in enumerate(
                    (q, s, c) for q in range(n_q) for s in range(n_sb)
                    for c in range(N_CHUNK)):
                t = cpool.tile([P, sb], F32R, tag=f"cb_{c}_{q}_{s}")
                dma_engs[j % 3].dma_start(
                    out=t, in_=cb_tiles[c, q][:, s * sb:(s + 1) * sb])
                cb_sb[c, q, s] = t

            # --- main loop over token tiles ----------------------------------
            BATCH = 8
            assert n_tiles % BATCH == 0 or n_tiles < BATCH
            batch = min(BATCH, n_tiles)
            tts = [t for _ in range(repeat) for t in range(n_tiles)]
            for bstart in range(0, len(tts), batch):
                btiles = tts[bstart:bstart + batch]
                nb = len(btiles)
                s_all = smpool.tile([P, nb, n_q], F32, tag="s_all")
                for bi, tt in enumerate(btiles):
                    xt = xpool.tile([P, N_CHUNK, P], F32R, tag="xt")
                    nc.sync.dma_start(out=xt, in_=x_tiles[tt])

                    qmax = smpool.tile([P, n_q], F32, tag="qmax")
                    scores = []
                    for q in range(n_q):
                        ps = pspool.tile([P, quarter], F32, tag="ps")
                        for c in range(N_CHUNK):
                            for s in range(n_sb):
                                nc.tensor.matmul(
                                    out=ps[:, s * sb:(s + 1) * sb],
                                    lhsT=xt[:, c, :],
                                    rhs=cb_sb[c, q, s][:, :],
                                    start=(c == 0),
                                    stop=False,
                                )
                        for s in range(n_sb):
                            nc.tensor.matmul(
                                out=ps[:, s * sb:(s + 1) * sb],
                                lhsT=sel_sb[q][:, :],
                                rhs=csq_sb[:, s * sb:(s + 1) * sb],
                                start=False,
                                stop=True,
                            )
                        # drain scores PSUM -> SBUF on the idle ACT engine;
                        # DVE reduces the SBUF copy so the PSUM bank is
                        # released by ACT alone (DVE runs backlogged and
                        # would stall PE's next accumulation group)
                        score = spool.tile([P, quarter], F32, tag="score")
                        nc.scalar.copy(score, ps)
                        nc.vector.reduce_max(
                            out=qmax[:, q:q + 1], in_=score,
                            axis=mybir.AxisListType.X)
                        scores.append(score)

                    # tau = gmax - DELTA
                    gmax = sm1pool.tile([P, 1], F32, tag="gmax")
                    nc.vector.reduce_max(
                        out=gmax, in_=qmax, axis=mybir.AxisListType.X)
                    tau = sm1pool.tile([P, 1], F32, tag="tau")
                    nc.vector.tensor_scalar_add(tau, gmax, -DELTA)

                    # pass 2: S_q = sum_k (score_k >= tau) * (BPACK + k_local)
                    for q in range(n_q):
                        dummy = sm1pool.tile(
                            [P, 1], F32, tag=f"dummy{min(q, 1)}")
                        nc.vector.scalar_tensor_tensor(
                            out=dummy.broadcast_to((P, quarter)),
                            in0=scores[q],
                            scalar=tau,
                            in1=iota_sb,
                            op0=mybir.AluOpType.is_ge,
                            op1=mybir.AluOpType.mult,
                            accum_out=s_all[:, bi, q:q + 1],
                        )

                # ---- batched decode over `nb` tiles ([P, nb*n_q] ops) -------
                nbq = nb * n_q
                sflat = s_all.rearrange("p b q -> p (b q)")
                t1 = sm1pool.tile([P, nbq], F32, tag="t1")
                nc.vector.tensor_scalar_mul(t1, sflat, 1.0 / BPACK)
                cnt_u = sm1pool.tile([P, nbq], mybir.dt.uint32, tag="cnt_u")
                nc.vector.tensor_copy(cnt_u, t1)
                countb = sm1pool.tile([P, nbq], F32, tag="countb")
                nc.vector.tensor_copy(countb, cnt_u)
                cntb = sm1pool.tile([P, nbq], F32, tag="cntb")
                nc.vector.tensor_scalar_mul(cntb, countb, BPACK)
                idx_local = sm1pool.tile([P, nbq], F32, tag="idx_local")
                nc.vector.tensor_sub(idx_local, sflat, cntb)
                # per-tile flags: sum counts over the quarter axis
                nc.vector.reduce_sum(
                    out=flags_sb[:, bstart % n_tiles:
                                 bstart % n_tiles + nb],
                    in_=countb.rearrange("p (b q) -> p b q", q=n_q),
                    axis=mybir.AxisListType.X)
                # global candidate index; select in-band quarters
                idxg = sm1pool.tile([P, nbq], F32, tag="idxg")
                nc.vector.tensor_add(idxg, idx_local, iota_nq_sb[:, :nbq])
                masked = sm1pool.tile([P, nbq], F32, tag="masked")
                nc.vector.scalar_tensor_tensor(
                    out=masked,
                    in0=countb,
                    scalar=0.5,
                    in1=idxg,
                    op0=mybir.AluOpType.is_ge,
                    op1=mybir.AluOpType.mult,
                )
                idxf = sm1pool.tile([P, nb], F32, tag="idxf")
                nc.vector.reduce_sum(
                    out=idxf,
                    in_=masked.rearrange("p (b q) -> p b q", q=n_q),
                    axis=mybir.AxisListType.X)
                idxc = sm1pool.tile([P, nb], F32, tag="idxc")
                nc.vector.tensor_scalar_min(idxc, idxf, float(k - 1))
                idxu = sm1pool.tile([P, nb], mybir.dt.uint32, tag="idxu")
                nc.vector.tensor_copy(idxu, idxc)

                # gather codebook rows and store, per tile in the batch
                for bi, tt in enumerate(btiles):
                    rows = opool.tile([P, D], F32, tag="rows")
                    nc.gpsimd.indirect_dma_start(
                        out=rows,
                        out_offset=None,
                        in_=codebook[:, :],
                        in_offset=bass.IndirectOffsetOnAxis(
                            ap=idxu[:, bi:bi + 1], axis=0),
                    )
                    nc.sync.dma_start(
                        out=out[tt * P:(tt + 1) * P, :], in_=rows)

            nc.sync.dma_start(out=out_flags[:, :], in_=flags_sb)

    return nc


def tf32_round(a, mant=MANT):
    """Round fp32 to `mant` explicit mantissa bits (round-to-nearest)."""
    ai = a.view(np.int32).astype(np.int64)
    shift = 23 - mant
    bias = 1 << (shift - 1)
    r = ((ai + bias) >> shift) << shift
    return r.astype(np.int32).view(np.float32)


def prep_core_inputs(x_core, shared, n_tiles):
    """Per-core input map. x_core: [n_tiles*P, D]."""
    xt = tf32_round(np.ascontiguousarray(
        x_core.reshape(n_tiles, P, N_CHUNK, P).transpose(0, 3, 2, 1)))
    return {"x_tiles": xt, **shared}


def prep_shared(codebook, k, quarter):
    n_q = k // quarter
    cb = np.ascontiguousarray(np.asarray(codebook, dtype=np.float32))
    cb2 = 2.0 * cb  # exact in fp32
    # cb_tiles[c, q, d, j] = cb2[q*quarter + j, c*128 + d]
    cb2_tiles = tf32_round(np.ascontiguousarray(
        cb2.reshape(n_q, quarter, N_CHUNK, P).transpose(2, 0, 3, 1)))
    csq = (cb * cb).sum(axis=1, dtype=np.float32)
    csqpack = np.zeros((2 * n_q, quarter), dtype=np.float32)
    selrows = np.zeros((n_q, 2 * n_q, P), dtype=np.float32)
    for q in range(n_q):
        seg = csq[q * quarter:(q + 1) * quarter]
        hi = tf32_round(-seg)
        lo = tf32_round(-seg - hi)
        csqpack[2 * q] = hi
        csqpack[2 * q + 1] = lo
        selrows[q, 2 * q, :] = 1.0
        selrows[q, 2 * q + 1, :] = 1.0
    iota_b_np = np.broadcast_to(
        (np.arange(quarter, dtype=np.float32) + BPACK)[None, :],
        (P, quarter)).copy()
    offs = np.tile(np.arange(n_q, dtype=np.float32) * quarter, 8)
    iota_nq_np = np.broadcast_to(offs[None, :], (P, len(offs))).copy()
    return {
        "cb_tiles": cb2_tiles,
        "csqpack": csqpack,
        "selrows": selrows,
        "iota_b": iota_b_np,
        "iota_nq": iota_nq_np,
        "codebook": cb,
    }


_NC_CACHE = {}


def _get_nc(key):
    if key not in _NC_CACHE:
        nc = build_bass(*key)
        nc.finalize()
        _NC_CACHE[key] = nc
    return _NC_CACHE[key]


def _host_rescue(out_full, flags_full, x, codebook):
    """Recompute flagged tokens exactly (float64)."""
    bad = np.flatnonzero(flags_full != 1.0)
    if len(bad) == 0:
        return out_full, 0
    xb = x[bad].astype(np.float64)
    cb64 = codebook.astype(np.float64)
    csq = (cb64 * cb64).sum(1)
    sc = 2.0 * (xb @ cb64.T) - csq[None, :]
    idx = sc.argmax(1)
    out_full[bad] = codebook[idx]
    return out_full, len(bad)


def kernel(x, codebook):
    from concourse.bass_utils import run_bass_kernel_spmd

    x = np.ascontiguousarray(np.asarray(x, dtype=np.float32))
    codebook = np.ascontiguousarray(np.asarray(codebook, dtype=np.float32))
    assert x.shape == (N_TOKENS, D) and codebook.shape == (K, D)

    nc = _get_nc((N_TILES_FULL, K, QUARTER_FULL))
    shared = prep_shared(codebook, K, QUARTER_FULL)

    in_maps = []
    for core in range(N_CORES):
        x_core = x[core * T_PER_CORE:(core + 1) * T_PER_CORE]
        in_maps.append(prep_core_inputs(x_core, shared, N_TILES_FULL))

    res = run_bass_kernel_spmd(nc, in_maps, list(range(N_CORES)))
    out_full = np.concatenate(
        [res.results[i]["out"] for i in range(N_CORES)], axis=0)
    # flags: [P, n_tiles] per core; token (core, tt*128+p) at [p, tt]
    flags_full = np.concatenate(
        [np.asarray(res.results[i]["out_flags"]).T.reshape(-1)
         for i in range(N_CORES)])
    out_full, n_rescued = _host_rescue(out_full, flags_full, x, codebook)
    return out_full
